# revision 5
# baseline (speedup 1.0000x reference)
"""Bahdanau-attention GRU decoder on 8 Trainium2 NeuronCores.

Data-parallel over batch: each of the 8 cores runs the full T=128 recurrence
for its 8 batches. No cross-core communication. PE matmul streams are bf16
(fp32 PSUM accumulate); recurrent state h is fp32.

kernel(**inputs) takes the full unsharded inputs (as produced by
reference.setup_inputs) and returns (hidden, outputs) matching reference().
"""
import os
import sys

sys.path.insert(0, "/opt/trn_rl_repo")

import numpy as np
import ml_dtypes

import concourse.bass as bass
import concourse.bacc as bacc
import concourse.tile as tile
import concourse.mybir as mybir
from concourse.bass_utils import run_bass_kernel_spmd

B, S, T, E, H = 64, 128, 128, 512, 1024
NCORES = 8
BC = B // NCORES          # 8 batches per core
HK = H // 128             # 8 h-chunks of 128
EK = E // 128             # 4 e-chunks
NEG = -1e9

F32 = mybir.dt.float32
BF16 = mybir.dt.bfloat16
AF = mybir.ActivationFunctionType
ALU = mybir.AluOpType
AX = mybir.AxisListType

T_STEPS = int(os.environ.get("BAHD_T", str(T)))

_BUILT = {}


def _chunk(dram_ap, k):
    """(K*128, N) dram tensor -> (128, N) chunk k."""
    return dram_ap.rearrange("(k p) n -> p k n", p=128)[:, k:k + 1, :].squeeze(1)


def _build(t_steps):
    nc = bacc.Bacc("TRN2", target_bir_lowering=False, debug=False,
                   num_devices=NCORES)

    def din(name, shape, dt=BF16):
        return nc.dram_tensor(name, shape, dt, kind="ExternalInput").ap()

    d_inpT = din("inpT", [E, BC * T])              # inputs^T  (E, (b,t))
    d_eht = din("eht", [H, BC * S])                # enc_hiddens^T (h, (b,s))
    d_eh = din("eh", [BC * S, H])                  # enc_hiddens ((b,s), h)
    d_wqT = din("wqT", [H, H])                     # W_query.T
    d_wkT = din("wkT", [H, H])                     # W_key.T
    d_whhT = din("whhT", [H, 3 * H])               # W_hh.T
    d_wihcT = din("wihcT", [H, 3 * H])             # W_ih[:, E:].T
    d_wihxT = din("wihxT", [E, 3 * H])             # W_ih[:, :E].T
    d_vblk = din("vblk", [128, 128])               # e-reduce stationaries
    d_emask = din("emask", [BC, 8 * S])            # 0 valid, -1e9 junk/masked
    d_bhhn = din("bhhn", [BC, H])                  # b_hh[2H:] replicated (bf16)
    d_gbias = din("gbias", [128, 3 * H])           # folded gate bias, replicated
    d_ident = din("ident", [128, 128])             # identity (bf16)
    d_h0 = din("h0", [BC, H], F32)
    d_h0T = din("h0T", [128, BC * HK])             # h0 transposed chunks (bf16)

    d_out = nc.dram_tensor("out", [t_steps, BC, H], F32,
                           kind="ExternalOutput").ap()

    with tile.TileContext(nc) as tc:
        with tc.tile_pool(name="sb", bufs=1) as P, \
             tc.tile_pool(name="rot", bufs=1) as R, \
             tc.tile_pool(name="ps", bufs=1, space="PSUM") as PS, \
             tc.tile_pool(name="dram", bufs=1, space="DRAM") as D:

            # ---------- resident tiles (per-partition KB in comments) ----------
            whh_sb = P.tile([128, HK * 3 * H], BF16, tag="whh")      # 48
            wihc_sb = P.tile([128, HK * 3 * H], BF16, tag="wihc")    # 48
            eh_sb = P.tile([128, BC * H], BF16, tag="eh")            # 16
            pk_sb = P.tile([128, HK * BC * S], BF16, tag="pk")       # 16
            vblk = P.tile([128, 128], BF16, tag="vblk")
            emask = P.tile([BC, 8 * S], BF16, tag="emask")           # 2
            bhhn = P.tile([BC, H], BF16, tag="bhhn")                 # 2
            ident = P.tile([128, 128], BF16, tag="ident")

            nc.sync.dma_start(whh_sb[:].rearrange("p (k n) -> p k n", k=HK),
                              d_whhT.rearrange("(k p) n -> p k n", p=128))
            nc.sync.dma_start(wihc_sb[:].rearrange("p (k n) -> p k n", k=HK),
                              d_wihcT.rearrange("(k p) n -> p k n", p=128))
            nc.sync.dma_start(eh_sb[:].rearrange("p (b n) -> p b n", b=BC),
                              d_eh.rearrange("(b p) n -> p b n", p=128))
            nc.sync.dma_start(vblk[:], d_vblk)
            nc.sync.dma_start(emask[:], d_emask)
            nc.sync.dma_start(bhhn[:], d_bhhn)
            nc.sync.dma_start(ident[:], d_ident)
            id8 = ident[0:8, 0:8]

            gxx_dram = D.tile([BC * T, 3 * H], BF16)

            # shared rotating tags:
            #   b2k : (*, <=1024) bf16 2KB  bufs=6            -> 12
            #   b6k : (*, 3072) bf16 6KB    bufs=2            -> 12
            #   f4k : (*, <=1024) f32 4KB   bufs=4 (scratch)  -> 16
            def b2k(shape, name):
                return R.tile(shape, BF16, tag="b2k", bufs=6, name=name)

            def b6k(shape, name):
                return R.tile(shape, BF16, tag="b6k", bufs=2, name=name)

            def f4k(name):
                return R.tile([BC, H], F32, tag="f4k", bufs=4, name=name)

            def ps2(name, shape=None, dt=F32):
                return PS.tile(shape or [BC, H], dt, tag="ps2", bufs=3,
                               name=name)

            # ---------- precompute: pk chunks ----------
            # pk_sb[p, (j,b,s)] = pk[b, s, 128j+p]
            for jg in range(4):
                pk_ps = [ps2(f"pkps{jg}_{jj}", [128, BC * S]) for jj in range(2)]
                for k in range(HK):
                    eht_k = b2k([128, BC * S], f"eht{jg}_{k}")
                    nc.sync.dma_start(eht_k[:], _chunk(d_eht, k))
                    wk_k = b2k([128, H], f"wk{jg}_{k}")
                    nc.sync.dma_start(wk_k[:], _chunk(d_wkT, k))
                    for jj in range(2):
                        j = jg * 2 + jj
                        for n in range(2):
                            nc.tensor.matmul(
                                pk_ps[jj][:, n * 512:(n + 1) * 512],
                                wk_k[:, j * 128:(j + 1) * 128],
                                eht_k[:, n * 512:(n + 1) * 512],
                                start=(k == 0), stop=(k == HK - 1))
                for jj in range(2):
                    j = jg * 2 + jj
                    nc.vector.tensor_copy(
                        pk_sb[:, j * BC * S:(j + 1) * BC * S], pk_ps[jj][:])

            # ---------- precompute: gxx = inputs @ W_ihx.T + gbias ----------
            gbias_sb = b6k([128, 3 * H], "gbias")
            nc.sync.dma_start(gbias_sb[:], d_gbias)
            for m in range(BC):          # m-tile == batch (T == 128)
                gps = [ps2(f"gxx{m}_{gi}", [128, H]) for gi in range(3)]
                for k in range(EK):
                    inT_k = b2k([128, BC * T], f"inT{m}_{k}")
                    nc.sync.dma_start(inT_k[:], _chunk(d_inpT, k))
                    wx_k = b6k([128, 3 * H], f"wx{m}_{k}")
                    nc.sync.dma_start(wx_k[:], _chunk(d_wihxT, k))
                    for gi in range(3):
                        for n in range(2):
                            col = gi * H + n * 512
                            nc.tensor.matmul(
                                gps[gi][:, n * 512:(n + 1) * 512],
                                inT_k[:, m * 128:(m + 1) * 128],
                                wx_k[:, col:col + 512],
                                start=(k == 0), stop=(k == EK - 1))
                gxs = b6k([128, 3 * H], f"gxs{m}")
                for gi in range(3):
                    nc.vector.tensor_tensor(
                        gxs[:, gi * H:(gi + 1) * H], gps[gi][:],
                        gbias_sb[:, gi * H:(gi + 1) * H], ALU.add)
                nc.sync.dma_start(gxx_dram[m * 128:(m + 1) * 128, :], gxs[:])

            # ---------- initial state ----------
            h_t = R.tile([BC, H], F32, tag="hst", bufs=2, name="h_init")
            nc.sync.dma_start(h_t[:], d_h0)
            hT = R.tile([128, BC * HK], BF16, tag="hT", bufs=2, name="hT_init")
            nc.sync.dma_start(hT[:], d_h0T)

            # ---------- recurrence ----------
            for t in range(t_steps):
                gxx_t = b6k([BC, 3 * H], f"gxxt{t}")
                nc.sync.dma_start(
                    gxx_t[:],
                    gxx_dram[:].rearrange("(b t) n -> t b n", t=T)[t:t + 1]
                    .squeeze(0))

                # --- q = h @ Wq.T -> qT chunks ---
                q_ps = ps2(f"qps{t}")
                for k in range(HK):
                    wq_k = b2k([128, H], f"wq{t}_{k}")
                    nc.sync.dma_start(wq_k[:], _chunk(d_wqT, k))
                    for n in range(2):
                        nc.tensor.matmul(
                            q_ps[:, n * 512:(n + 1) * 512],
                            hT[:, k * BC:(k + 1) * BC],
                            wq_k[:, n * 512:(n + 1) * 512],
                            start=(k == 0), stop=(k == HK - 1))
                q_sb = b2k([BC, H], f"qsb{t}")
                nc.vector.tensor_copy(q_sb[:], q_ps[:])
                qT_ps = PS.tile([128, BC * HK], BF16, tag="tps", bufs=2,
                                name=f"qTps{t}")
                for j in range(HK):
                    nc.tensor.transpose(qT_ps[:, j * BC:(j + 1) * BC],
                                        q_sb[:, j * 128:(j + 1) * 128], id8)
                qT = R.tile([128, BC * HK], BF16, tag="qT", bufs=2,
                            name=f"qT{t}")
                nc.vector.tensor_copy(qT[:], qT_ps[:])

                # --- attention middle; gh_r interleaved ---
                e_ps = ps2(f"eps{t}", [BC, 8 * S])
                ghp = [ps2(f"ghps{t}_{gi}") for gi in range(3)]
                for j in range(HK):
                    tin = b2k([128, BC * S], f"tin{t}_{j}")
                    nc.vector.tensor_tensor(
                        tin[:].rearrange("p (b s) -> p b s", b=BC),
                        pk_sb[:, j * BC * S:(j + 1) * BC * S]
                        .rearrange("p (b s) -> p b s", b=BC),
                        qT[:, j * BC:(j + 1) * BC].unsqueeze(2)
                        .broadcast_to((128, BC, S)),
                        ALU.add)
                    nc.scalar.activation(tin[:], tin[:], AF.Tanh)
                    for g in range(2):
                        nc.tensor.matmul(
                            e_ps[:, g * 512:(g + 1) * 512],
                            vblk[:, (2 * j + g) * 8:(2 * j + g + 1) * 8],
                            tin[:, g * 512:(g + 1) * 512],
                            start=(j == 0), stop=(j == HK - 1))
                    for n in range(2):    # gh gate 0 (r)
                        nc.tensor.matmul(
                            ghp[0][:, n * 512:(n + 1) * 512],
                            hT[:, j * BC:(j + 1) * BC],
                            whh_sb[:, j * 3 * H + n * 512:
                                   j * 3 * H + (n + 1) * 512],
                            start=(j == 0), stop=(j == HK - 1))

                gh_sb = [b2k([BC, H], f"ghsb{t}_{gi}") for gi in range(3)]
                nc.vector.tensor_copy(gh_sb[0][:], ghp[0][:])

                # --- softmax (junk/source mask folded in) ---
                e_m = f4k(f"em{t}")
                nc.vector.tensor_tensor(e_m[:, 0:8 * S], e_ps[:], emask[:],
                                        ALU.add)
                mx = R.tile([BC, 1], F32, tag="mx", bufs=2, name=f"mx{t}")
                nc.vector.tensor_reduce(mx[:], e_m[:, 0:8 * S], AX.X, ALU.max)
                nmx = R.tile([BC, 1], F32, tag="nmx", bufs=2, name=f"nmx{t}")
                nc.vector.tensor_scalar_mul(nmx[:], mx[:], -1.0)
                ssum = R.tile([BC, 1], F32, tag="ssum", bufs=2, name=f"ss{t}")
                nc.scalar.activation(e_m[:, 0:8 * S], e_m[:, 0:8 * S], AF.Exp,
                                     bias=nmx[:, 0:1], accum_out=ssum[:])
                rs = R.tile([BC, 1], F32, tag="rs", bufs=2, name=f"rs{t}")
                nc.vector.reciprocal(rs[:], ssum[:])
                alphas = b2k([BC, 8 * S], f"al{t}")
                nc.vector.tensor_scalar(alphas[:], e_m[:, 0:8 * S],
                                        rs[:, 0:1], None, ALU.mult)

                # gh gate 1 (z) — fills PE while softmax runs on DVE/ACT
                for j in range(HK):
                    for n in range(2):
                        col = j * 3 * H + H + n * 512
                        nc.tensor.matmul(
                            ghp[1][:, n * 512:(n + 1) * 512],
                            hT[:, j * BC:(j + 1) * BC],
                            whh_sb[:, col:col + 512],
                            start=(j == 0), stop=(j == HK - 1))

                nc.vector.tensor_copy(gh_sb[1][:], ghp[1][:])

                # alphaT chunks -> block-diag ctx stationaries (junk cols = 0)
                aT_ps = PS.tile([128, BC * 8], BF16, tag="tps", bufs=2,
                                name=f"aTps{t}")
                for c in range(8):
                    nc.tensor.transpose(aT_ps[:, c * 8:(c + 1) * 8],
                                        alphas[:, c * 128:(c + 1) * 128], id8)
                A_bf = R.tile([128, BC * 8], BF16, tag="Abf", bufs=2,
                              name=f"Abf{t}")
                nc.vector.tensor_copy(A_bf[:], aT_ps[:])

                # --- ctx = alphas @ EH ;  gh gate 2 (n) interleaved ---
                ctx_ps = ps2(f"ctxps{t}")
                for b in range(BC):
                    for n in range(2):
                        nc.tensor.matmul(
                            ctx_ps[:, n * 512:(n + 1) * 512],
                            A_bf[:, b * 8:(b + 1) * 8],
                            eh_sb[:, b * H + n * 512:b * H + (n + 1) * 512],
                            start=(b == 0), stop=(b == BC - 1))
                for j in range(HK):
                    for n in range(2):
                        col = j * 3 * H + 2 * H + n * 512
                        nc.tensor.matmul(
                            ghp[2][:, n * 512:(n + 1) * 512],
                            hT[:, j * BC:(j + 1) * BC],
                            whh_sb[:, col:col + 512],
                            start=(j == 0), stop=(j == HK - 1))
                nc.vector.tensor_copy(gh_sb[2][:], ghp[2][:])
                ctx_bf = b2k([BC, H], f"ctxbf{t}")
                nc.vector.tensor_copy(ctx_bf[:], ctx_ps[:])
                cT_ps = PS.tile([128, BC * HK], BF16, tag="tps", bufs=2,
                                name=f"cTps{t}")
                for j in range(HK):
                    nc.tensor.transpose(cT_ps[:, j * BC:(j + 1) * BC],
                                        ctx_bf[:, j * 128:(j + 1) * 128], id8)
                ctxT = R.tile([128, BC * HK], BF16, tag="ctxT", bufs=2,
                              name=f"ctxT{t}")
                nc.vector.tensor_copy(ctxT[:], cT_ps[:])

                # --- per gate: gx matmul + gate math ---
                gates = {}
                for gi in range(3):      # r, z, n
                    gx_ps = ps2(f"gxps{t}_{gi}")
                    for k in range(HK):
                        for n in range(2):
                            col = k * 3 * H + gi * H + n * 512
                            nc.tensor.matmul(
                                gx_ps[:, n * 512:(n + 1) * 512],
                                ctxT[:, k * BC:(k + 1) * BC],
                                wihc_sb[:, col:col + 512],
                                start=(k == 0), stop=(k == HK - 1))
                    tsum = f4k(f"ts{t}_{gi}")
                    nc.vector.tensor_tensor(
                        tsum[:], gx_ps[:], gxx_t[:, gi * H:(gi + 1) * H],
                        ALU.add)
                    if gi < 2:
                        pre = f4k(f"pre{t}_{gi}")
                        nc.vector.tensor_tensor(pre[:], tsum[:],
                                                gh_sb[gi][:], ALU.add)
                        gate = R.tile([BC, H], F32, tag=f"g{gi}", bufs=2,
                                      name=f"g{gi}_{t}")
                        nc.scalar.activation(gate[:], pre[:], AF.Sigmoid)
                        gates[gi] = gate
                    else:
                        t1 = f4k(f"t1_{t}")
                        nc.vector.tensor_tensor(t1[:], gh_sb[2][:], bhhn[:],
                                                ALU.add)
                        t2 = f4k(f"t2_{t}")
                        nc.vector.tensor_tensor(t2[:], gates[0][:], t1[:],
                                                ALU.mult)
                        t3 = f4k(f"t3_{t}")
                        nc.vector.tensor_tensor(t3[:], tsum[:], t2[:], ALU.add)
                        gate = R.tile([BC, H], F32, tag="gn", bufs=2,
                                      name=f"gn_{t}")
                        nc.scalar.activation(gate[:], t3[:], AF.Tanh)
                        gates[2] = gate

                # --- h' = n + z*(h - n) ---
                hm = f4k(f"hm{t}")
                nc.vector.tensor_tensor(hm[:], h_t[:], gates[2][:],
                                        ALU.subtract)
                zm = f4k(f"zm{t}")
                nc.vector.tensor_tensor(zm[:], gates[1][:], hm[:], ALU.mult)
                h_new = R.tile([BC, H], F32, tag="hst", bufs=2,
                               name=f"h{t + 1}")
                nc.vector.tensor_tensor(h_new[:], gates[2][:], zm[:], ALU.add)
                nc.sync.dma_start(d_out[t:t + 1].squeeze(0), h_new[:])

                h_t = h_new
                if t + 1 < t_steps:
                    hbf = b2k([BC, H], f"hbf{t}")
                    nc.vector.tensor_copy(hbf[:], h_new[:])
                    hT_ps = PS.tile([128, BC * HK], BF16, tag="tps", bufs=2,
                                    name=f"hTps{t}")
                    for j in range(HK):
                        nc.tensor.transpose(hT_ps[:, j * BC:(j + 1) * BC],
                                            hbf[:, j * 128:(j + 1) * 128], id8)
                    hT = R.tile([128, BC * HK], BF16, tag="hT", bufs=2,
                                name=f"hT{t + 1}")
                    nc.vector.tensor_copy(hT[:], hT_ps[:])

    nc.compile()
    return nc


def _prep_core(c, inputs, encoder_hiddens, encoder_finals, src_mask,
               W_key, W_query, v_att, W_ih, W_hh, b_ih, b_hh):
    bf = ml_dtypes.bfloat16
    sl = slice(c * BC, (c + 1) * BC)
    inp = np.ascontiguousarray(inputs[sl])                # (BC,T,E)
    eh = np.ascontiguousarray(encoder_hiddens[sl])        # (BC,S,H)
    h0 = np.ascontiguousarray(encoder_finals[0, sl]).astype(np.float32)
    msk = src_mask[sl]

    inpT = inp.transpose(2, 0, 1).reshape(E, BC * T)
    eht = eh.transpose(2, 0, 1).reshape(H, BC * S)
    ehr = eh.reshape(BC * S, H)

    vblk = np.zeros((128, 128), np.float32)
    for j in range(HK):
        vj = v_att[j * 128:(j + 1) * 128]
        for g in range(2):
            for i in range(4):
                b = 4 * g + i
                vblk[:, (2 * j + g) * 8 + b] = vj

    emask = np.full((BC, 8 * S), NEG, np.float32)
    for b in range(BC):
        emask[b, 128 * b:128 * (b + 1)] = np.where(msk[b], 0.0, NEG)

    gbias = np.concatenate([b_ih[:H] + b_hh[:H],
                            b_ih[H:2 * H] + b_hh[H:2 * H],
                            b_ih[2 * H:]]).astype(np.float32)
    gbias_rep = np.tile(gbias[None, :], (128, 1))
    bhhn_rep = np.tile(b_hh[2 * H:][None, :].astype(np.float32), (BC, 1))

    h0T = h0.T.reshape(HK, 128, BC).transpose(1, 0, 2).reshape(128, BC * HK)

    return {
        "inpT": inpT.astype(bf),
        "eht": eht.astype(bf),
        "eh": ehr.astype(bf),
        "wqT": W_query.T.astype(bf),
        "wkT": W_key.T.astype(bf),
        "whhT": W_hh.T.astype(bf),
        "wihcT": W_ih[:, E:].T.astype(bf),
        "wihxT": W_ih[:, :E].T.astype(bf),
        "vblk": vblk.astype(bf),
        "emask": emask.astype(bf),
        "bhhn": bhhn_rep.astype(bf),
        "gbias": gbias_rep.astype(bf),
        "ident": np.eye(128, dtype=np.float32).astype(bf),
        "h0": h0,
        "h0T": h0T.astype(bf),
    }


def kernel(inputs, encoder_hiddens, encoder_finals, src_mask,
           W_key, W_query, v_att, W_ih, W_hh, b_ih, b_hh, **extra):
    args = [np.asarray(a) if np.asarray(a).dtype == bool
            else np.asarray(a, dtype=np.float32) for a in
            (inputs, encoder_hiddens, encoder_finals, src_mask,
             W_key, W_query, v_att, W_ih, W_hh, b_ih, b_hh)]
    t_steps = T_STEPS
    if t_steps not in _BUILT:
        _BUILT[t_steps] = _build(t_steps)
    nc = _BUILT[t_steps]

    in_maps = [_prep_core(c, *args) for c in range(NCORES)]
    res = run_bass_kernel_spmd(nc, in_maps, core_ids=list(range(NCORES)))
    kernel._last_results = res

    outs = np.concatenate(
        [res.results[c]["out"].transpose(1, 0, 2) for c in range(NCORES)],
        axis=0)                                        # (B, t_steps, H)
    hidden = outs[:, -1][None].copy()
    return hidden, outs


# revision 14
# speedup vs baseline: 503.5536x; 503.5536x over previous
"""Bahdanau-attention GRU decoder on 8 Trainium2 NeuronCores.

Data-parallel over batch: each of the 8 cores runs the full T=128 recurrence
for its 8 batches. No cross-core communication. PE matmul streams are bf16
(fp32 PSUM accumulate); recurrent state h is fp32.

kernel(**inputs) takes the full unsharded inputs (as produced by
reference.setup_inputs) and returns (hidden, outputs) matching reference().
"""
import os
import sys

sys.path.insert(0, "/opt/trn_rl_repo")

import numpy as np
import ml_dtypes

import concourse.bass as bass
import concourse.bacc as bacc
import concourse.tile as tile
import concourse.mybir as mybir
from concourse.bass_utils import run_bass_kernel_spmd

B, S, T, E, H = 64, 128, 128, 512, 1024
NCORES = 8
BC = B // NCORES          # 8 batches per core
HK = H // 128             # 8 h-chunks of 128
EK = E // 128             # 4 e-chunks
NEG = -1e9

F32 = mybir.dt.float32
BF16 = mybir.dt.bfloat16
AF = mybir.ActivationFunctionType
ALU = mybir.AluOpType
AX = mybir.AxisListType

T_STEPS = int(os.environ.get("BAHD_T", str(T)))

_BUILT = {}


def _chunk(dram_ap, k):
    """(K*128, N) dram tensor -> (128, N) chunk k."""
    return dram_ap.rearrange("(k p) n -> p k n", p=128)[:, k:k + 1, :].squeeze(1)


def _build(t_steps):
    nc = bacc.Bacc("TRN2", target_bir_lowering=False, debug=False,
                   num_devices=NCORES)

    def din(name, shape, dt=BF16):
        return nc.dram_tensor(name, shape, dt, kind="ExternalInput").ap()

    d_inpT = din("inpT", [E, BC * T])              # inputs^T  (E, (b,t))
    d_eht = din("eht", [H, BC * S])                # enc_hiddens^T (h, (b,s))
    d_eh = din("eh", [BC * S, H])                  # enc_hiddens ((b,s), h)
    d_wqT = din("wqT", [H, H])                     # W_query.T
    d_wkT = din("wkT", [H, H])                     # W_key.T
    d_whhT = din("whhT", [H, 3 * H])               # W_hh.T
    d_wihcT = din("wihcT", [H, 3 * H])             # W_ih[:, E:].T
    d_wihxT = din("wihxT", [E, 3 * H])             # W_ih[:, :E].T
    d_vblk = din("vblk", [128, 128])               # e-reduce stationaries
    d_ajunk = din("ajunk", [128, BC * 8])          # 0/1 junk zeroing pattern
    d_emask = din("emask", [BC, 8 * S])            # 0 valid, -1e9 junk/masked
    d_bhhn = din("bhhn", [BC, H])                  # b_hh[2H:] replicated (bf16)
    d_gbias = din("gbias", [128, 3 * H])           # folded gate bias, replicated
    d_ident = din("ident", [128, 128])             # identity (bf16)
    d_idf = din("idf", [128, 8], F32)              # identity cols (f32)
    d_h0 = din("h0", [BC, H], F32)
    d_h0T = din("h0T", [128, BC * HK])             # h0 transposed chunks (bf16)

    d_out = nc.dram_tensor("out", [t_steps, BC, H], F32,
                           kind="ExternalOutput").ap()

    with tile.TileContext(nc) as tc:
        with tc.tile_pool(name="sb", bufs=1) as P, \
             tc.tile_pool(name="rot", bufs=1) as R, \
             tc.tile_pool(name="ps", bufs=1, space="PSUM") as PS, \
             tc.tile_pool(name="dram", bufs=1, space="DRAM") as D:

            # ---------- resident tiles (per-partition KB in comments) ----------
            whh_sb = P.tile([128, HK * 3 * H], BF16, tag="whh")      # 48
            wihc_sb = P.tile([128, HK * 3 * H], BF16, tag="wihc")    # 48
            eh_sb = P.tile([128, BC * H], BF16, tag="eh")            # 16
            pk_sb = P.tile([128, HK * BC * S], BF16, tag="pk")       # 16
            vblk = P.tile([128, 128], BF16, tag="vblk")
            ajunk = P.tile([128, BC * 8], BF16, tag="ajunk")
            emask = P.tile([BC, 8 * S], BF16, tag="emask")           # 2
            bhhn = P.tile([BC, H], BF16, tag="bhhn")                 # 2
            ident = P.tile([128, 128], BF16, tag="ident")
            idf = P.tile([128, 8], F32, tag="idf")

            nc.sync.dma_start(whh_sb[:].rearrange("p (k n) -> p k n", k=HK),
                              d_whhT.rearrange("(k p) n -> p k n", p=128))
            nc.sync.dma_start(wihc_sb[:].rearrange("p (k n) -> p k n", k=HK),
                              d_wihcT.rearrange("(k p) n -> p k n", p=128))
            nc.sync.dma_start(eh_sb[:].rearrange("p (b n) -> p b n", b=BC),
                              d_eh.rearrange("(b p) n -> p b n", p=128))
            nc.sync.dma_start(vblk[:], d_vblk)
            nc.sync.dma_start(ajunk[:], d_ajunk)
            nc.sync.dma_start(emask[:], d_emask)
            nc.sync.dma_start(bhhn[:], d_bhhn)
            nc.sync.dma_start(ident[:], d_ident)
            nc.sync.dma_start(idf[:], d_idf)
            id8 = ident[0:8, 0:8]

            gxx_dram = D.tile([BC * T, 3 * H], BF16)

            # shared rotating tags:
            #   b2k : (*, <=1024) bf16 2KB  bufs=6            -> 12
            #   b6k : (*, 3072) bf16 6KB    bufs=2            -> 12
            #   f4k : (*, <=1024) f32 4KB   bufs=4 (scratch)  -> 16
            def b2k(shape, name):
                return R.tile(shape, BF16, tag="b2k", bufs=6, name=name)

            def b6k(shape, name):
                return R.tile(shape, BF16, tag="b6k", bufs=2, name=name)

            def f4k(name):
                return R.tile([BC, H], F32, tag="f4k", bufs=4, name=name)

            def ps2(name, shape=None, dt=F32):
                return PS.tile(shape or [BC, H], dt, tag="ps2", bufs=3,
                               name=name)

            # ---------- precompute: pk chunks ----------
            # pk_sb[p, (j,b,s)] = pk[b, s, 128j+p]
            for jg in range(4):
                pk_ps = [ps2(f"pkps{jg}_{jj}", [128, BC * S]) for jj in range(2)]
                for k in range(HK):
                    eht_k = b2k([128, BC * S], f"eht{jg}_{k}")
                    nc.sync.dma_start(eht_k[:], _chunk(d_eht, k))
                    wk_k = b2k([128, H], f"wk{jg}_{k}")
                    nc.sync.dma_start(wk_k[:], _chunk(d_wkT, k))
                    for jj in range(2):
                        j = jg * 2 + jj
                        for n in range(2):
                            nc.tensor.matmul(
                                pk_ps[jj][:, n * 512:(n + 1) * 512],
                                wk_k[:, j * 128:(j + 1) * 128],
                                eht_k[:, n * 512:(n + 1) * 512],
                                start=(k == 0), stop=(k == HK - 1))
                for jj in range(2):
                    j = jg * 2 + jj
                    nc.vector.tensor_copy(
                        pk_sb[:, j * BC * S:(j + 1) * BC * S], pk_ps[jj][:])

            # ---------- precompute: gxx = inputs @ W_ihx.T + gbias ----------
            gbias_sb = b6k([128, 3 * H], "gbias")
            nc.sync.dma_start(gbias_sb[:], d_gbias)
            for m in range(BC):          # m-tile == batch (T == 128)
                gps = [ps2(f"gxx{m}_{gi}", [128, H]) for gi in range(3)]
                for k in range(EK):
                    inT_k = b2k([128, BC * T], f"inT{m}_{k}")
                    nc.sync.dma_start(inT_k[:], _chunk(d_inpT, k))
                    wx_k = b6k([128, 3 * H], f"wx{m}_{k}")
                    nc.sync.dma_start(wx_k[:], _chunk(d_wihxT, k))
                    for gi in range(3):
                        for n in range(2):
                            col = gi * H + n * 512
                            nc.tensor.matmul(
                                gps[gi][:, n * 512:(n + 1) * 512],
                                inT_k[:, m * 128:(m + 1) * 128],
                                wx_k[:, col:col + 512],
                                start=(k == 0), stop=(k == EK - 1))
                gxs = b6k([128, 3 * H], f"gxs{m}")
                for gi in range(3):
                    nc.vector.tensor_tensor(
                        gxs[:, gi * H:(gi + 1) * H], gps[gi][:],
                        gbias_sb[:, gi * H:(gi + 1) * H], ALU.add)
                nc.sync.dma_start(gxx_dram[m * 128:(m + 1) * 128, :], gxs[:])

            # ---------- initial state ----------
            h_t = R.tile([BC, H], F32, tag="hst", bufs=2, name="h_init")
            nc.sync.dma_start(h_t[:], d_h0)
            hT = R.tile([128, BC * HK], BF16, tag="hT", bufs=2, name="hT_init")
            nc.sync.dma_start(hT[:], d_h0T)

            # ---------- recurrence ----------
            q_ps_carry = None
            for t in range(t_steps):
                gxx_t = b6k([BC, 3 * H], f"gxxt{t}")
                nc.sync.dma_start(
                    gxx_t[:],
                    gxx_dram[:].rearrange("(b t) n -> t b n", t=T)[t:t + 1]
                    .squeeze(0))

                # --- q = h @ Wq.T (k 0-7 here for t=0; carried from the
                #     previous step's tail otherwise) ---
                if q_ps_carry is None:
                    q_ps = ps2(f"qps{t}")
                    for k in range(HK):
                        wq_k = b2k([128, H], f"wq{t}_{k}")
                        nc.sync.dma_start(wq_k[:], _chunk(d_wqT, k))
                        for n in range(2):
                            nc.tensor.matmul(
                                q_ps[:, n * 512:(n + 1) * 512],
                                hT[:, k * BC:(k + 1) * BC],
                                wq_k[:, n * 512:(n + 1) * 512],
                                start=(k == 0), stop=(k == HK - 1))
                else:
                    q_ps = q_ps_carry
                q_sb = b2k([BC, H], f"qsb{t}")
                nc.scalar.copy(q_sb[:], q_ps[:])
                qT_ps = PS.tile([128, BC * HK], BF16, tag="aux", bufs=1,
                                name=f"qTps{t}")
                for j in range(HK):
                    nc.tensor.transpose(qT_ps[:, j * BC:(j + 1) * BC],
                                        q_sb[:, j * 128:(j + 1) * 128], id8)
                qT = R.tile([128, BC * HK], BF16, tag="qT", bufs=2,
                            name=f"qT{t}")
                nc.vector.tensor_copy(qT[:], qT_ps[:])

                # --- DVE broadcast-adds + ACT tanh ---
                tins = []
                for j in range(HK):
                    tin = b2k([128, BC * S], f"tin{t}_{j}")
                    nc.vector.tensor_tensor(
                        tin[:].rearrange("p (b s) -> p b s", b=BC),
                        pk_sb[:, j * BC * S:(j + 1) * BC * S]
                        .rearrange("p (b s) -> p b s", b=BC),
                        qT[:, j * BC:(j + 1) * BC].unsqueeze(2)
                        .broadcast_to((128, BC, S)),
                        ALU.add)
                    nc.scalar.activation(tin[:], tin[:], AF.Tanh)
                    tins.append(tin)

                # --- gh r,z matmuls stream into the gx psums ---
                gx_ps = [ps2(f"gxps{t}_{gi}") for gi in range(2)]
                for gi in range(2):
                    for j in range(HK):
                        for n in range(2):
                            col = j * 3 * H + gi * H + n * 512
                            nc.tensor.matmul(
                                gx_ps[gi][:, n * 512:(n + 1) * 512],
                                hT[:, j * BC:(j + 1) * BC],
                                whh_sb[:, col:col + 512],
                                start=(j == 0), stop=False)

                # --- e-reduce; first half of ghn interleaved ---
                e_ps = PS.tile([BC, 8 * S], F32, tag="aux", bufs=1,
                               name=f"eps{t}")
                ghn_ps = ps2(f"ghn{t}")
                for j in range(HK):
                    for g in range(2):
                        nc.tensor.matmul(
                            e_ps[:, g * 512:(g + 1) * 512],
                            vblk[:, (2 * j + g) * 8:(2 * j + g + 1) * 8],
                            tins[j][:, g * 512:(g + 1) * 512],
                            start=(j == 0), stop=(j == HK - 1))
                    if j < 4:
                        for n in range(2):
                            col = j * 3 * H + 2 * H + n * 512
                            nc.tensor.matmul(
                                ghn_ps[:, n * 512:(n + 1) * 512],
                                hT[:, j * BC:(j + 1) * BC],
                                whh_sb[:, col:col + 512],
                                start=(j == 0), stop=False)

                e_m = f4k(f"em{t}")
                nc.vector.tensor_tensor(e_m[:, 0:8 * S], e_ps[:], emask[:],
                                        ALU.add)
                ssum = R.tile([BC, 1], F32, tag="ssum", bufs=2, name=f"ss{t}")
                nc.scalar.activation(e_m[:, 0:8 * S], e_m[:, 0:8 * S], AF.Exp,
                                     accum_out=ssum[:])
                rs = R.tile([BC, 1], F32, tag="rs", bufs=2, name=f"rs{t}")
                nc.vector.reciprocal(rs[:], ssum[:])
                alphas = b2k([BC, 8 * S], f"al{t}")
                nc.vector.tensor_scalar(alphas[:], e_m[:, 0:8 * S],
                                        rs[:, 0:1], None, ALU.mult)

                # second half of ghn fills the softmax window
                for j in range(4, HK):
                    for n in range(2):
                        col = j * 3 * H + 2 * H + n * 512
                        nc.tensor.matmul(
                            ghn_ps[:, n * 512:(n + 1) * 512],
                            hT[:, j * BC:(j + 1) * BC],
                            whh_sb[:, col:col + 512],
                            start=False, stop=False)
                for n in range(2):
                    nc.tensor.matmul(
                        ghn_ps[:, n * 512:(n + 1) * 512],
                        id8, bhhn[:, n * 512:(n + 1) * 512],
                        start=False, stop=True)
                ghn_sb = b2k([BC, H], f"ghnsb{t}")
                nc.scalar.copy(ghn_sb[:], ghn_ps[:])

                # alphaT chunks -> block-diag ctx stationaries (junk cols
                # are exp-junk * rs -- NOT zero -> zero them via the
                # junk-masked copy below)  [junk cols = alphas cols outside
                # 128b:128(b+1) per row b; after transpose chunk c keeps
                # only col c valid -- other cols must be 0. exp(0)=1 junk
                # would corrupt ctx, so subtract the junk pattern:]
                aT_ps = PS.tile([128, BC * 8], BF16, tag="aux", bufs=1,
                                name=f"aTps{t}")
                for c in range(8):
                    nc.tensor.transpose(aT_ps[:, c * 8:(c + 1) * 8],
                                        alphas[:, c * 128:(c + 1) * 128], id8)
                A_bf = R.tile([128, BC * 8], BF16, tag="Abf", bufs=2,
                              name=f"Abf{t}")
                nc.vector.tensor_copy(A_bf[:], aT_ps[:])

                # --- ctx = alphas @ EH  (BC, H) ---
                ctx_ps = ps2(f"ctxps{t}")
                for b in range(BC):
                    for n in range(2):
                        nc.tensor.matmul(
                            ctx_ps[:, n * 512:(n + 1) * 512],
                            A_bf[:, b * 8:(b + 1) * 8],
                            eh_sb[:, b * H + n * 512:b * H + (n + 1) * 512],
                            start=(b == 0), stop=(b == BC - 1))
                ctx_bf = b2k([BC, H], f"ctxbf{t}")
                nc.scalar.copy(ctx_bf[:], ctx_ps[:])
                cT_ps = PS.tile([128, BC * HK], BF16, tag="aux", bufs=1,
                                name=f"cTps{t}")
                for j in range(HK):
                    nc.tensor.transpose(cT_ps[:, j * BC:(j + 1) * BC],
                                        ctx_bf[:, j * 128:(j + 1) * 128], id8)
                ctxT = R.tile([128, BC * HK], BF16, tag="ctxT", bufs=2,
                              name=f"ctxT{t}")
                nc.vector.tensor_copy(ctxT[:], cT_ps[:])

                # --- finish r,z gate psums: ctx part + gxx fold ---
                taus = {}
                for gi in range(2):
                    for k in range(HK):
                        for n in range(2):
                            col = k * 3 * H + gi * H + n * 512
                            nc.tensor.matmul(
                                gx_ps[gi][:, n * 512:(n + 1) * 512],
                                ctxT[:, k * BC:(k + 1) * BC],
                                wihc_sb[:, col:col + 512],
                                start=False, stop=False)
                    for n in range(2):
                        nc.tensor.matmul(
                            gx_ps[gi][:, n * 512:(n + 1) * 512],
                            id8, gxx_t[:, gi * H + n * 512:gi * H + (n + 1) * 512],
                            start=False, stop=True)
                    tau = f4k(f"tau{t}_{gi}")
                    nc.scalar.activation(tau[:], gx_ps[gi][:], AF.Tanh,
                                         scale=0.5)
                    gate = R.tile([BC, H], F32, tag=f"g{gi}", bufs=2,
                                  name=f"g{gi}_{t}")
                    nc.vector.tensor_scalar(gate[:], tau[:], 0.5, 0.5,
                                            ALU.mult, ALU.add)
                    taus[gi] = gate
                    if gi == 0:
                        t2 = f4k(f"t2_{t}")
                        nc.vector.tensor_tensor(t2[:], gate[:], ghn_sb[:],
                                                ALU.mult)
                    else:
                        zi = f4k(f"zi{t}")
                        nc.vector.tensor_scalar(zi[:], gate[:], -1.0, 1.0,
                                                ALU.mult, ALU.add)
                        zh = f4k(f"zh{t}")

                # n gate per half; tail pipelined with PE (separate psum
                # tiles per half to avoid same-tensor serialization)
                h_new = R.tile([BC, H], F32, tag="hst", bufs=2,
                               name=f"h{t + 1}")
                hT_ps = PS.tile([128, BC * HK], F32, tag="aux", bufs=1,
                                name=f"hTps{t}") if t + 1 < t_steps else None
                hT_next = R.tile([128, BC * HK], BF16, tag="hT", bufs=2,
                                 name=f"hT{t + 1}") if t + 1 < t_steps else None
                q_ps_carry = ps2(f"qps{t + 1}") if t + 1 < t_steps else None
                if q_ps_carry is not None:
                    wq_next = []
                    for k in range(HK):
                        wq_k = b2k([128, H], f"wq{t + 1}_{k}")
                        nc.sync.dma_start(wq_k[:], _chunk(d_wqT, k))
                        wq_next.append(wq_k)
                for n in range(2):
                    sl = slice(n * 512, (n + 1) * 512)
                    gxn = ps2(f"gxn{t}_{n}", [BC, 512])
                    for k in range(HK):
                        col = k * 3 * H + 2 * H + n * 512
                        nc.tensor.matmul(
                            gxn[:],
                            ctxT[:, k * BC:(k + 1) * BC],
                            wihc_sb[:, col:col + 512],
                            start=(k == 0), stop=False)
                    nc.tensor.matmul(
                        gxn[:],
                        id8, gxx_t[:, 2 * H + n * 512:2 * H + (n + 1) * 512],
                        start=False, stop=True)
                    t3 = R.tile([BC, 512], F32, tag="half", bufs=3,
                                name=f"t3_{t}_{n}")
                    nc.vector.tensor_tensor(t3[:], gxn[:], t2[:, sl], ALU.add)
                    nh = R.tile([BC, 512], F32, tag="half", bufs=3,
                                name=f"nh{t}_{n}")
                    nc.scalar.activation(nh[:], t3[:], AF.Tanh)
                    if n == 0:
                        nc.vector.tensor_tensor(zh[:], taus[1][:], h_t[:],
                                                ALU.mult)
                    t4 = R.tile([BC, 512], F32, tag="half", bufs=3,
                                name=f"t4_{t}_{n}")
                    nc.vector.tensor_tensor(t4[:], nh[:], zi[:, sl], ALU.mult)
                    nc.vector.tensor_tensor(h_new[:, sl], t4[:], zh[:, sl],
                                            ALU.add)
                    if hT_ps is not None:
                        for j in range(n * 4, n * 4 + 4):
                            nc.tensor.transpose(
                                hT_ps[:, j * BC:(j + 1) * BC],
                                h_new[:, j * 128:(j + 1) * 128], idf[0:8, 0:8])
                        nc.vector.tensor_copy(
                            hT_next[:, n * 4 * BC:(n + 1) * 4 * BC],
                            hT_ps[:, n * 4 * BC:(n + 1) * 4 * BC])
                        for k in range(n * 4, n * 4 + 4):
                            for m in range(2):
                                nc.tensor.matmul(
                                    q_ps_carry[:, m * 512:(m + 1) * 512],
                                    hT_next[:, k * BC:(k + 1) * BC],
                                    wq_next[k][:, m * 512:(m + 1) * 512],
                                    start=(k == 0), stop=(k == HK - 1))
                nc.sync.dma_start(d_out[t:t + 1].squeeze(0), h_new[:])

                h_t = h_new
                if hT_next is not None:
                    hT = hT_next

    nc.compile()
    return nc


def _prep_core(c, inputs, encoder_hiddens, encoder_finals, src_mask,
               W_key, W_query, v_att, W_ih, W_hh, b_ih, b_hh):
    bf = ml_dtypes.bfloat16
    sl = slice(c * BC, (c + 1) * BC)
    inp = np.ascontiguousarray(inputs[sl])                # (BC,T,E)
    eh = np.ascontiguousarray(encoder_hiddens[sl])        # (BC,S,H)
    h0 = np.ascontiguousarray(encoder_finals[0, sl]).astype(np.float32)
    msk = src_mask[sl]

    inpT = inp.transpose(2, 0, 1).reshape(E, BC * T)
    eht = eh.transpose(2, 0, 1).reshape(H, BC * S)
    ehr = eh.reshape(BC * S, H)

    vblk = np.zeros((128, 128), np.float32)
    for j in range(HK):
        vj = v_att[j * 128:(j + 1) * 128]
        for g in range(2):
            for i in range(4):
                b = 4 * g + i
                vblk[:, (2 * j + g) * 8 + b] = vj

    emask = np.full((BC, 8 * S), NEG, np.float32)
    for b in range(BC):
        emask[b, 128 * b:128 * (b + 1)] = np.where(msk[b], 0.0, NEG)
    ajunk_np = np.zeros((128, BC * 8), np.float32)
    for c in range(8):
        ajunk_np[:, c * 8 + c] = 1.0

    gbias = np.concatenate([b_ih[:H] + b_hh[:H],
                            b_ih[H:2 * H] + b_hh[H:2 * H],
                            b_ih[2 * H:]]).astype(np.float32)
    gbias_rep = np.tile(gbias[None, :], (128, 1))
    bhhn_rep = np.tile(b_hh[2 * H:][None, :].astype(np.float32), (BC, 1))

    h0T = h0.T.reshape(HK, 128, BC).transpose(1, 0, 2).reshape(128, BC * HK)

    return {
        "inpT": inpT.astype(bf),
        "eht": eht.astype(bf),
        "eh": ehr.astype(bf),
        "wqT": W_query.T.astype(bf),
        "wkT": W_key.T.astype(bf),
        "whhT": W_hh.T.astype(bf),
        "wihcT": W_ih[:, E:].T.astype(bf),
        "wihxT": W_ih[:, :E].T.astype(bf),
        "vblk": vblk.astype(bf),
        "ajunk": ajunk_np.astype(bf),
        "emask": emask.astype(bf),
        "bhhn": bhhn_rep.astype(bf),
        "gbias": gbias_rep.astype(bf),
        "ident": np.eye(128, dtype=np.float32).astype(bf),
        "idf": np.eye(128, 8, dtype=np.float32),
        "h0": h0,
        "h0T": h0T.astype(bf),
    }


def kernel(inputs, encoder_hiddens, encoder_finals, src_mask,
           W_key, W_query, v_att, W_ih, W_hh, b_ih, b_hh, **extra):
    args = [np.asarray(a) if np.asarray(a).dtype == bool
            else np.asarray(a, dtype=np.float32) for a in
            (inputs, encoder_hiddens, encoder_finals, src_mask,
             W_key, W_query, v_att, W_ih, W_hh, b_ih, b_hh)]
    t_steps = T_STEPS
    if t_steps not in _BUILT:
        _BUILT[t_steps] = _build(t_steps)
    nc = _BUILT[t_steps]

    in_maps = [_prep_core(c, *args) for c in range(NCORES)]
    res = run_bass_kernel_spmd(nc, in_maps, core_ids=list(range(NCORES)))
    kernel._last_results = res

    outs = np.concatenate(
        [res.results[c]["out"].transpose(1, 0, 2) for c in range(NCORES)],
        axis=0)                                        # (B, t_steps, H)
    hidden = outs[:, -1][None].copy()
    return hidden, outs
